# revision 17
# baseline (speedup 1.0000x reference)
"""DKVMN forward kernel for 8 Trainium2 NeuronCores (Bass/Tile).

Problem (per reference):
  B=64, T=200, D=256 (dim_s), M=64 (size_m), NUM_SKILLS=1000.
  k = k_emb[skills]; v = v_emb[skills + 1000*responses]
  w = softmax(k @ Mk^T)             (B,T,M)
  e = sigmoid(v @ eW^T + eb); a = tanh(v @ aW^T + ab)
  scan over t:  read_t = w_t^T Mv_t ;  Mv_{t+1} = Mv_t*(1 - w_t e_t^T) + w_t a_t^T
  f = tanh([reads, k] @ fW^T + fb);  p = sigmoid(f @ pW^T + pb)[..., 0]
  return p[:, 1:]

Sharding: data-parallel over batch, 8 batches/core, params replicated.

Per-core layout decisions:
  * tokens are indexed i = 256*b + t  (t padded 200->256) so that the
    token-major gather tile (128 x 16 x 256) has token i at partition i%128,
    group i//128 -- affine in (b,t): p = t%128, g = 2b + t//128.
  * feed-forward matmuls run feature-major (contraction dim on partitions).
  * scan state Mv is bf16, stored (128 x 1024): partition = 8*dq + b,
    column = 64*dr + m, where d = 16*dq + dr.  The w-broadcast tiles for
    ALL steps are prebuilt on the PE (selector B8) into W_all (bf16).
    e/a are staged in bf16 with each value duplicated into adjacent pairs
    (E2/A2) so the broadcast-along-m operand still has an innermost
    step-1 AP, letting the DVE run its 2x bf16 mode.
"""

import numpy as np
import ml_dtypes

import concourse.bass as bass
import concourse.mybir as mybir
import concourse.tile as tile
from concourse import bacc
from concourse.bass_utils import run_bass_kernel_spmd

F32 = mybir.dt.float32
BF16 = mybir.dt.bfloat16
I32 = mybir.dt.int32
ALU = mybir.AluOpType
ACTF = mybir.ActivationFunctionType
AXL = mybir.AxisListType

B, T, D, M, V = 64, 200, 256, 64, 1000
NCORES = 8
BL = B // NCORES          # 8 batches per core
TP = 256                  # padded time
NTOK = BL * TP            # 2048 tokens / core
NG = NTOK // 128          # 16 token groups
RSPL = 16                 # dr split: DVE gets [0,RSPL), gpsimd [RSPL,16)
                          # (16 = all-DVE; measured fastest with the bf16 scan)


def build_nc(rspl=RSPL, steps=T, enable_scan=True, repeat=1, pair=True, no_wall=False, no_e2load=False, read_gp=False):
    nc = bacc.Bacc("TRN2", target_bir_lowering=False, enable_partition_id=False)

    # ---- external inputs -------------------------------------------------
    idx_k_d = nc.dram_tensor("idx_k", [128, NG], I32, kind="ExternalInput")
    idx_v_d = nc.dram_tensor("idx_v", [128, NG], I32, kind="ExternalInput")
    k_emb_d = nc.dram_tensor("k_emb", [V, D], F32, kind="ExternalInput")
    v_emb_d = nc.dram_tensor("v_emb", [2 * V, D], F32, kind="ExternalInput")
    mkT_d = nc.dram_tensor("mkT", [D, M], F32, kind="ExternalInput")
    ewT_d = nc.dram_tensor("ewT", [D, D], F32, kind="ExternalInput")
    awT_d = nc.dram_tensor("awT", [D, D], F32, kind="ExternalInput")
    fwT_d = nc.dram_tensor("fwT", [2 * D, D], F32, kind="ExternalInput")
    pwT_d = nc.dram_tensor("pwT", [D, 1], F32, kind="ExternalInput")
    eb_d = nc.dram_tensor("eb", [D, 1], F32, kind="ExternalInput")
    ab_d = nc.dram_tensor("ab", [D, 1], F32, kind="ExternalInput")
    fb_d = nc.dram_tensor("fb", [D, 1], F32, kind="ExternalInput")
    pb_d = nc.dram_tensor("pb", [1, 1], F32, kind="ExternalInput")
    mv0_d = nc.dram_tensor("mv0", [128, 1024], BF16, kind="ExternalInput")
    b8_d = nc.dram_tensor("b8sel", [BL, 128], F32, kind="ExternalInput")
    id_d = nc.dram_tensor("ident", [128, 128], F32, kind="ExternalInput")

    # ---- DRAM scratch (layout bounces) ----------------------------------
    e_scan_d = nc.dram_tensor("e_scan_x", [128, 16 * TP], BF16, kind="Internal")
    a_scan_d = nc.dram_tensor("a_scan_x", [128, 16 * TP], BF16, kind="Internal")
    rT_d = nc.dram_tensor("rT_x", [D, NTOK], F32, kind="Internal")
    w8_d = nc.dram_tensor("w8_x", [BL, T, M], F32, kind="Internal")

    # ---- output ----------------------------------------------------------
    p_out_d = nc.dram_tensor("p_out", [1, NTOK], F32, kind="ExternalOutput")

    with tile.TileContext(nc) as tc:
        for _rep in range(max(1, repeat)):
            with tc.tile_pool(name="persist", bufs=1) as pp:
                # weights / constants that live for the whole kernel
                mkT_sb = pp.tile([128, 2, M], F32)
                nc.sync.dma_start(
                    out=mkT_sb, in_=mkT_d.ap().rearrange("(h p) m -> p h m", p=128)
                )
                ewT_sb = pp.tile([128, 2, D], F32)
                nc.sync.dma_start(
                    out=ewT_sb, in_=ewT_d.ap().rearrange("(h p) n -> p h n", p=128)
                )
                awT_sb = pp.tile([128, 2, D], F32)
                nc.sync.dma_start(
                    out=awT_sb, in_=awT_d.ap().rearrange("(h p) n -> p h n", p=128)
                )
                fwT_sb = pp.tile([128, 4, D], F32)
                nc.sync.dma_start(
                    out=fwT_sb, in_=fwT_d.ap().rearrange("(q p) n -> p q n", p=128)
                )
                pwT_sb = pp.tile([128, 2, 1], F32)
                nc.sync.dma_start(
                    out=pwT_sb, in_=pwT_d.ap().rearrange("(h p) n -> p h n", p=128)
                )
                eb_sb = pp.tile([128, 2], F32)
                nc.sync.dma_start(
                    out=eb_sb, in_=eb_d.ap().rearrange("(h p) n -> p (h n)", p=128)
                )
                ab_sb = pp.tile([128, 2], F32)
                nc.sync.dma_start(
                    out=ab_sb, in_=ab_d.ap().rearrange("(h p) n -> p (h n)", p=128)
                )
                fb_sb = pp.tile([128, 2], F32)
                nc.sync.dma_start(
                    out=fb_sb, in_=fb_d.ap().rearrange("(h p) n -> p (h n)", p=128)
                )
                pb_sb = pp.tile([1, 1], F32)
                nc.sync.dma_start(out=pb_sb, in_=pb_d.ap())
                b8_sb = pp.tile([BL, 128], F32)
                nc.sync.dma_start(out=b8_sb, in_=b8_d.ap())
                id_sb = pp.tile([128, 128], F32)
                nc.sync.dma_start(out=id_sb, in_=id_d.ap())

                mv_sb = pp.tile([128, 1024], BF16)
                nc.sync.dma_start(out=mv_sb, in_=mv0_d.ap())
                if read_gp or rspl == 16:
                    mv2_sb = pp.tile([128, 1024], BF16)
                else:
                    mv2_sb = None

                # long-lived activations (scan inputs/outputs), bf16 with
                # duplicated pairs for the broadcast-along-m operands
                E2_sb = pp.tile([128, 16 * TP, 2], BF16)
                A2_sb = pp.tile([128, 16 * TP, 2], BF16)
                W_all = pp.tile([128, T * M], BF16)   # replicated w, all steps
                reads_sb = pp.tile([128, 16 * TP], F32)
                nc.vector.memset(reads_sb, 0.0)

                # ---------------- gather + transpose + ff ---------------------
                with (
                    tc.tile_pool(name="gath", bufs=2) as gp,
                    tc.tile_pool(name="ffsb", bufs=1) as fp,
                    tc.tile_pool(name="ps", bufs=2, space="PSUM") as psp,
                    tc.tile_pool(name="pst", bufs=2, space="PSUM") as pst,
                ):
                    idxk_sb = fp.tile([128, NG], I32)
                    nc.sync.dma_start(out=idxk_sb, in_=idx_k_d.ap())
                    idxv_sb = fp.tile([128, NG], I32)
                    nc.sync.dma_start(out=idxv_sb, in_=idx_v_d.ap())

                    kT_sb = pp.tile([128, 2, NTOK], F32)
                    vT_sb = fp.tile([128, 2, NTOK], F32)

                    for idx_sb, emb_d, dst in (
                        (idxk_sb, k_emb_d, kT_sb),
                        (idxv_sb, v_emb_d, vT_sb),
                    ):
                        tok = gp.tile([128, NG, D], F32, tag="tok")
                        for g in range(NG):
                            nc.gpsimd.indirect_dma_start(
                                out=tok[:, g, :],
                                out_offset=None,
                                in_=emb_d.ap(),
                                in_offset=bass.IndirectOffsetOnAxis(
                                    ap=idx_sb[:, g : g + 1], axis=0
                                ),
                            )
                        for g in range(NG):
                            for h in range(2):
                                tp_ps = pst.tile([128, 128], F32, tag="tp")
                                nc.tensor.transpose(
                                    tp_ps, tok[:, g, 128 * h : 128 * (h + 1)], id_sb
                                )
                                eng = nc.scalar if (g + h) % 2 else nc.vector
                                if eng is nc.scalar:
                                    eng.copy(dst[:, h, 128 * g : 128 * (g + 1)], tp_ps)
                                else:
                                    eng.tensor_copy(
                                        dst[:, h, 128 * g : 128 * (g + 1)], tp_ps
                                    )

                    # scores^T = Mk @ k^T : (M x NTOK)
                    sc_sb = fp.tile([M, NTOK], F32)
                    for ch in range(4):
                        sc_ps = psp.tile([M, 512], F32, tag="sc")
                        for h in range(2):
                            nc.tensor.matmul(
                                sc_ps,
                                mkT_sb[:, h, :],
                                kT_sb[:, h, 512 * ch : 512 * (ch + 1)],
                                start=(h == 0),
                                stop=(h == 1),
                            )
                        nc.scalar.copy(sc_sb[:, 512 * ch : 512 * (ch + 1)], sc_ps)

                    # transpose scores -> token-major (128 x NG x M) in PSUM
                    sct_ps = psp.tile([128, NG, M], F32, tag="sct", bufs=1)
                    for g in range(NG):
                        nc.tensor.transpose(
                            sct_ps[:, g, :],
                            sc_sb[:, 128 * g : 128 * (g + 1)],
                            id_sb[:M, :M],
                        )

                    # softmax over m (free dim); no max-subtraction (|scores|<~10)
                    ex_sb = fp.tile([128, NG, M], F32)
                    nc.scalar.activation(ex_sb, sct_ps, ACTF.Exp)
                    sum_sb = fp.tile([128, NG], F32)
                    nc.vector.tensor_reduce(sum_sb, ex_sb, axis=AXL.X, op=ALU.add)
                    rec_sb = fp.tile([128, NG], F32)
                    nc.vector.reciprocal(rec_sb, sum_sb)
                    nc.vector.tensor_tensor(
                        ex_sb,
                        ex_sb,
                        rec_sb.unsqueeze(2).broadcast_to([128, NG, M]),
                        op=ALU.mult,
                    )  # ex_sb now holds w (token-major)

                    # w8[b, t, m] = w(token 256b+t): token at (p=t%128, g=2b+t//128)
                    for b in range(BL):
                        for th in range(2):
                            tlen = 128 if th == 0 else T - 128
                            nc.sync.dma_start(
                                out=w8_d.ap()[b : b + 1, 128 * th : 128 * th + tlen, :],
                                in_=ex_sb[0:tlen, 2 * b + th, :],
                            )

                    # W_all[p, t*M+m] = w[t, b(p)%8, m] (partition-replicated w
                    # for every step, bf16) via the B8 selector on the PE
                    w8_flat = w8_d.ap().rearrange("b t m -> b (t m)")
                    for ch in range(0 if no_wall else T * M // 512):
                        w8_sb = gp.tile([BL, 512], F32, tag="w8ch")
                        nc.sync.dma_start(
                            out=w8_sb, in_=w8_flat[:, 512 * ch : 512 * (ch + 1)]
                        )
                        w_ps = psp.tile([128, 512], F32, tag="ea")
                        nc.tensor.matmul(w_ps, b8_sb, w8_sb, start=True, stop=True)
                        nc.scalar.copy(W_all[:, 512 * ch : 512 * (ch + 1)], w_ps)

                    # e = sigmoid(eW v + eb), a = tanh(aW v + ab)  (feature-major,
                    # bf16, bounced to DRAM scan layout)
                    for wsb, bsb, func, dram in (
                        (ewT_sb, eb_sb, ACTF.Sigmoid, e_scan_d),
                        (awT_sb, ab_sb, ACTF.Tanh, a_scan_d),
                    ):
                        dv = dram.ap().rearrange(
                            "(q b) (r t) -> q r b t", b=BL, t=TP
                        )  # (16, 16, 8, TP)
                        for ho in range(2):
                            xT_sb = fp.tile([128, NTOK], BF16, tag="ea_half")
                            for ch in range(4):
                                ea_ps = psp.tile([128, 512], F32, tag="ea")
                                for hi in range(2):
                                    nc.tensor.matmul(
                                        ea_ps,
                                        wsb[:, hi, 128 * ho : 128 * (ho + 1)],
                                        vT_sb[:, hi, 512 * ch : 512 * (ch + 1)],
                                        start=(hi == 0),
                                        stop=(hi == 1),
                                    )
                                nc.scalar.activation(
                                    xT_sb[:, 512 * ch : 512 * (ch + 1)],
                                    ea_ps,
                                    func,
                                    bias=bsb[:, ho : ho + 1],
                                )
                            for dq in range(8 * ho, 8 * ho + 8):
                                prow = (dq % 8) * 16
                                nc.sync.dma_start(
                                    out=dv[dq], in_=xT_sb[prow : prow + 16, :]
                                )
                    # load back in scan layout, then duplicate each value into
                    # adjacent pairs on-chip (strided copies on idle engines)
                    for i, (dram, dst) in enumerate(
                        () if no_e2load else ((e_scan_d, E2_sb), (a_scan_d, A2_sb))
                    ):
                        ea_sb = fp.tile([128, 16 * TP], BF16, tag=f"ea_lin{i}")
                        nc.sync.dma_start(out=ea_sb, in_=dram.ap())
                        nc.vector.tensor_copy(dst[:, :, 0], ea_sb)
                        nc.scalar.copy(dst[:, :, 1], ea_sb)

                # ---------------- the scan ------------------------------------
                mv3 = mv_sb.rearrange("p (r m) -> p r m", m=M)
                mv4 = mv_sb.rearrange("p (r m2 two) -> p r m2 two", two=2, m2=M // 2)
                E2v = E2_sb.rearrange("p (r t) two -> p r t two", t=TP)
                A2v = A2_sb.rearrange("p (r t) two -> p r t two", t=TP)
                Wv = W_all.rearrange("p (t m) -> p t m", m=M)
                Rv = reads_sb.rearrange("p (r t) -> p r t", t=TP)
                if read_gp:
                    # reads (P = w*Mv and its m-sum) fully on GPSIMD via
                    # scalar_tensor_tensor accum; update chain on DVE with
                    # ping-pong state buffers (no write-after-read hazard)
                    with tc.tile_pool(name="scan", bufs=3) as sp:
                        bufs = (mv_sb, mv2_sb)
                        for t in range(steps if enable_scan else 0):
                            cur = bufs[t % 2]
                            nxt = bufs[(t + 1) % 2]
                            cur3 = cur.rearrange("p (r m) -> p r m", m=M)
                            cur4 = cur.rearrange(
                                "p (r m2 two) -> p r m2 two", two=2, m2=M // 2
                            )
                            nxt3 = nxt.rearrange("p (r m) -> p r m", m=M)
                            wbv = (
                                Wv[:, t, :].unsqueeze(1).broadcast_to([128, 16, M])
                            )
                            if t > 0:
                                Pg = sp.tile([128, 16, M], BF16, tag="Pg")
                                nc.gpsimd.tensor_tensor(Pg, cur3, wbv, op=ALU.mult)
                                nc.vector.tensor_reduce(
                                    Rv[:, :, t], Pg, axis=AXL.X, op=ALU.add
                                )
                            if t == steps - 1:
                                continue
                            ebc = (
                                E2v[:, :, t, :]
                                .unsqueeze(2)
                                .broadcast_to([128, 16, M // 2, 2])
                            )
                            abc = (
                                A2v[:, :, t, :]
                                .unsqueeze(2)
                                .broadcast_to([128, 16, M // 2, 2])
                            )
                            Qt = sp.tile([128, 16, M // 2, 2], BF16, tag="Q")
                            nc.vector.tensor_tensor(Qt, ebc, cur4, op=ALU.mult)
                            Zt = sp.tile([128, 16, M // 2, 2], BF16, tag="Z")
                            nc.vector.tensor_tensor(Zt, abc, Qt, op=ALU.subtract)
                            Zt3 = Zt.rearrange("p r m2 two -> p r (m2 two)")
                            ZWt = sp.tile([128, 16, M], BF16, tag="ZW")
                            nc.vector.tensor_tensor(ZWt, wbv, Zt3, op=ALU.mult)
                            nc.vector.tensor_tensor(nxt3, cur3, ZWt, op=ALU.add)
                elif rspl == 16:
                    # all-DVE scan with fixed scratch tiles (every dep is
                    # same-engine program order) and reduces batched over
                    # K_RED steps through a P ring to amortize the per-op
                    # overhead and cut instruction count
                    K_RED = 7
                    Pring = pp.tile([128, K_RED, 16, M], BF16)
                    Qf = pp.tile([128, 16, M // 2, 2], BF16)
                    Zf = pp.tile([128, 16, M // 2, 2], BF16)
                    ZWf = pp.tile([128, 16, M], BF16)
                    Zt3 = Zf.rearrange("p r m2 two -> p r (m2 two)")
                    pend = []

                    def flush_reads():
                        if not pend:
                            return
                        t0, L = pend[0], len(pend)
                        nc.vector.tensor_reduce(
                            Rv[:, :, t0 : t0 + L].rearrange("p r t -> p t r"),
                            Pring[:, 0:L].rearrange("p k r m -> p (k r) m"),
                            axis=AXL.X,
                            op=ALU.add,
                        )
                        pend.clear()

                    bufs_mv = (mv_sb, mv2_sb)
                    for t in range(steps if enable_scan else 0):
                        cur = bufs_mv[t % 2]
                        nxt = bufs_mv[(t + 1) % 2]
                        cur3 = cur.rearrange("p (r m) -> p r m", m=M)
                        cur4 = cur.rearrange(
                            "p (r m2 two) -> p r m2 two", two=2, m2=M // 2
                        )
                        nxt3 = nxt.rearrange("p (r m) -> p r m", m=M)
                        wbv = Wv[:, t, :].unsqueeze(1).broadcast_to([128, 16, M])
                        if t > 0:
                            # P on the otherwise-idle GPSIMD; DVE only syncs
                            # with it at the 1-per-8-steps batched reduce
                            nc.gpsimd.tensor_tensor(
                                Pring[:, len(pend)], cur3, wbv, op=ALU.mult
                            )
                            pend.append(t)
                            if len(pend) == K_RED:
                                flush_reads()
                        if t == steps - 1:
                            flush_reads()
                            continue
                        ebc = (
                            E2v[:, :, t, :]
                            .unsqueeze(2)
                            .broadcast_to([128, 16, M // 2, 2])
                        )
                        abc = (
                            A2v[:, :, t, :]
                            .unsqueeze(2)
                            .broadcast_to([128, 16, M // 2, 2])
                        )
                        nc.vector.tensor_tensor(Qf, ebc, cur4, op=ALU.mult)
                        nc.vector.tensor_tensor(Zf, abc, Qf, op=ALU.subtract)
                        nc.vector.tensor_tensor(ZWf, wbv, Zt3, op=ALU.mult)
                        nc.vector.tensor_tensor(nxt3, cur3, ZWf, op=ALU.add)
                elif True:
                  with tc.tile_pool(name="scan", bufs=3) as sp:
                    for t in range(steps if enable_scan else 0):
                        for eng, lo, hi in (
                            (nc.vector, 0, rspl),
                            (nc.gpsimd, rspl, 16),
                        ):
                            nr = hi - lo
                            if nr <= 0:
                                continue
                            wbv = (
                                Wv[:, t, :]
                                .unsqueeze(1)
                                .broadcast_to([128, 16, M])[:, lo:hi, :]
                            )
                            mvs = mv3[:, lo:hi, :]
                            if t > 0:
                                Pt = sp.tile([128, nr, M], BF16, tag=f"P{lo}")
                                eng.tensor_tensor(Pt, mvs, wbv, op=ALU.mult)
                                nc.vector.tensor_reduce(
                                    Rv[:, lo:hi, t], Pt, axis=AXL.X, op=ALU.add
                                )
                            if t == steps - 1:
                                continue
                            mvs4 = mv4[:, lo:hi, :, :]
                            if pair:
                                ebc = (
                                    E2v[:, lo:hi, t, :]
                                    .unsqueeze(2)
                                    .broadcast_to([128, nr, M // 2, 2])
                                )
                                abc = (
                                    A2v[:, lo:hi, t, :]
                                    .unsqueeze(2)
                                    .broadcast_to([128, nr, M // 2, 2])
                                )
                            else:
                                ebc = (
                                    E2v[:, lo:hi, t, 0:1]
                                    .unsqueeze(2)
                                    .broadcast_to([128, nr, M // 2, 2])
                                )
                                abc = (
                                    A2v[:, lo:hi, t, 0:1]
                                    .unsqueeze(2)
                                    .broadcast_to([128, nr, M // 2, 2])
                                )
                            Qt = sp.tile([128, nr, M // 2, 2], BF16, tag=f"Q{lo}")
                            eng.tensor_tensor(Qt, ebc, mvs4, op=ALU.mult)
                            Zt = sp.tile([128, nr, M // 2, 2], BF16, tag=f"Z{lo}")
                            eng.tensor_tensor(Zt, abc, Qt, op=ALU.subtract)
                            Zt3 = Zt.rearrange("p r m2 two -> p r (m2 two)")
                            ZWt = sp.tile([128, nr, M], BF16, tag=f"ZW{lo}")
                            eng.tensor_tensor(ZWt, wbv, Zt3, op=ALU.mult)
                            eng.tensor_tensor(mvs, mvs, ZWt, op=ALU.add)

                # ---------------- reads -> feature-major; f; p ----------------
                with (
                    tc.tile_pool(name="post", bufs=1) as qp,
                    tc.tile_pool(name="postps", bufs=2, space="PSUM") as qpp,
                ):
                    rdv = rT_d.ap().rearrange(
                        "(q r) (b t) -> q b r t", q=16, t=TP
                    )  # (16, 8, 16, TP)
                    for dq in range(16):
                        nc.sync.dma_start(
                            out=rdv[dq], in_=reads_sb[BL * dq : BL * dq + BL, :]
                        )
                    rT_sb = qp.tile([128, 2, NTOK], F32)
                    nc.sync.dma_start(
                        out=rT_sb, in_=rT_d.ap().rearrange("(h p) n -> p h n", p=128)
                    )
                    kT2_sb = kT_sb

                    # f^T = tanh(fW [reads; k] + fb)
                    quarters = (
                        rT_sb[:, 0, :],
                        rT_sb[:, 1, :],
                        kT2_sb[:, 0, :],
                        kT2_sb[:, 1, :],
                    )
                    fT_sb = qp.tile([128, 2, NTOK], F32)
                    for ho in range(2):
                        for ch in range(4):
                            f_ps = qpp.tile([128, 512], F32, tag="f")
                            for qi in range(4):
                                nc.tensor.matmul(
                                    f_ps,
                                    fwT_sb[:, qi, 128 * ho : 128 * (ho + 1)],
                                    quarters[qi][:, 512 * ch : 512 * (ch + 1)],
                                    start=(qi == 0),
                                    stop=(qi == 3),
                                )
                            nc.scalar.activation(
                                fT_sb[:, ho, 512 * ch : 512 * (ch + 1)],
                                f_ps,
                                ACTF.Tanh,
                                bias=fb_sb[:, ho : ho + 1],
                            )

                    # p = sigmoid(pW f + pb)
                    p_sb = qp.tile([1, NTOK], F32)
                    for ch in range(4):
                        p_ps = qpp.tile([1, 512], F32, tag="p")
                        for h in range(2):
                            nc.tensor.matmul(
                                p_ps,
                                pwT_sb[:, h, :],
                                fT_sb[:, h, 512 * ch : 512 * (ch + 1)],
                                start=(h == 0),
                                stop=(h == 1),
                            )
                        nc.scalar.activation(
                            p_sb[:, 512 * ch : 512 * (ch + 1)],
                            p_ps,
                            ACTF.Sigmoid,
                            bias=pb_sb,
                        )
                    nc.sync.dma_start(out=p_out_d.ap(), in_=p_sb)

    nc.compile()
    return nc


def _wrap_idx(flat):
    """token j lives at idxs[j % 128, j // 128] (int32)."""
    arr = np.zeros((128, NG), np.int32)
    j = np.arange(NTOK)
    arr[j % 128, j // 128] = flat.astype(np.int32)
    return arr


def prepare_in_maps(inputs):
    skills = np.asarray(inputs["skills"])
    responses = np.asarray(inputs["responses"])
    x = (skills + V * responses).astype(np.int64)

    Mk = np.asarray(inputs["Mk"], np.float32)
    Mv0 = np.asarray(inputs["Mv0"], np.float32)
    eW = np.asarray(inputs["eW"], np.float32)
    aW = np.asarray(inputs["aW"], np.float32)
    fW = np.asarray(inputs["fW"], np.float32)
    pW = np.asarray(inputs["pW"], np.float32)

    # scan-layout Mv0: [8*dq+b, 64*dr+m] = Mv0[m, 16*dq+dr]
    mv0_sc = np.ascontiguousarray(
        np.broadcast_to(
            Mv0.T.reshape(16, 1, 16 * M), (16, BL, 16 * M)
        ).reshape(128, 1024)
    ).astype(ml_dtypes.bfloat16)
    b8 = np.zeros((BL, 128), np.float32)
    for b in range(BL):
        b8[b, b::BL] = 1.0

    common = {
        "k_emb": np.asarray(inputs["k_emb"], np.float32),
        "v_emb": np.asarray(inputs["v_emb"], np.float32),
        "mkT": np.ascontiguousarray(Mk.T),
        "ewT": np.ascontiguousarray(eW.T),
        "awT": np.ascontiguousarray(aW.T),
        "fwT": np.ascontiguousarray(fW.T),
        "pwT": np.ascontiguousarray(pW.T),
        "eb": np.asarray(inputs["eb"], np.float32).reshape(D, 1),
        "ab": np.asarray(inputs["ab"], np.float32).reshape(D, 1),
        "fb": np.asarray(inputs["fb"], np.float32).reshape(D, 1),
        "pb": np.asarray(inputs["pb"], np.float32).reshape(1, 1),
        "mv0": mv0_sc,
        "b8sel": b8,
        "ident": np.eye(128, dtype=np.float32),
    }

    in_maps = []
    for c in range(NCORES):
        rows = slice(c * BL, (c + 1) * BL)
        sk = np.zeros((BL, TP), np.int64)
        sk[:, :T] = skills[rows]
        xv = np.zeros((BL, TP), np.int64)
        xv[:, :T] = x[rows]
        m = dict(common)
        m["idx_k"] = _wrap_idx(sk.reshape(-1))
        m["idx_v"] = _wrap_idx(xv.reshape(-1))
        in_maps.append(m)
    return in_maps


_CACHE = {}


def run_on_hw(inputs, trace=False):
    if "nc" not in _CACHE:
        _CACHE["nc"] = build_nc()
    nc = _CACHE["nc"]
    in_maps = prepare_in_maps(inputs)
    res = run_bass_kernel_spmd(
        nc, in_maps, core_ids=list(range(NCORES)), trace=trace
    )
    outs = []
    for c in range(NCORES):
        p = res.results[c]["p_out"].reshape(BL, TP)
        outs.append(p[:, 1:T])
    out = np.concatenate(outs, axis=0).astype(np.float32)
    return out, res


def kernel(**inputs):
    out, _ = run_on_hw(inputs, trace=False)
    return out
